# revision 8
# baseline (speedup 1.0000x reference)
"""GATv2 message-passing network (3 layers + sum-pool + MLP) on 8 trn2 NeuronCores.

Strategy: shard dst-nodes across 8 cores (contiguous ranges balanced by edge
count). Layer-0 source projections are computed REPLICATED on every core from
the full input feature matrix (PE is idle, features are only 5MB) — no
layer-0 AllGather. For layers 1-2 the fs table is AllGathered in G_CH chunks
that are software-pipelined into the previous layer's edge loop: as soon as a
chunk's worth of dst-tiles finish their attention update, their next-layer
projections run on the PE and that chunk's AllGather fires, hiding collective
latency behind the (DVE-bound) edge-softmax compute. Per-edge work runs in a
[128 dst, d, 256] layout: gather fs rows (one SWDGE gather per tile), add fd
broadcast, Prelu, a-weighted head reduce, edge softmax (normalization folded
to the end), alpha-weighted value sum. Pooling is a one-hot matmul into PSUM
+ AllReduce; the classifier is replicated on every core in f32.
"""
import sys
from contextlib import ExitStack

sys.path.insert(0, "/opt/trn_rl_repo")

import numpy as np
import ml_dtypes

BF = ml_dtypes.bfloat16
NC = 8
N_NODES = 20000
N_EDGES = 320000
IN_DIM = 128
HID = 256
HEADS = 8
DH = 32
LAYERS = 3
G = 64
OUT_DIM = 10
P = 128

G_CH = 4          # AllGather chunks per layer
STREAM_TILES = 8   # featT streaming chunk (global tiles per DMA)

_CACHE = {}


def _preprocess(src, dst):
    deg = np.bincount(dst, minlength=N_NODES)
    order = np.argsort(dst, kind="stable")
    src_by_dst = src[order]
    starts = np.zeros(N_NODES + 1, np.int64)
    np.cumsum(deg, out=starts[1:])

    csum = starts[1:]
    bounds = [0]
    for c in range(1, NC):
        i = int(np.searchsorted(csum, N_EDGES * c / NC))
        bounds.append(i + 1)
    bounds.append(N_NODES)
    shards = [(bounds[i], bounds[i + 1]) for i in range(NC)]
    node_counts = [b - a for a, b in shards]

    NB = ((max(node_counts) + 1 + 127) // 128) * 128
    while NB % (G_CH * 128):
        NB += 128
    T = NB // 128
    CH = NB // G_CH
    TC = T // G_CH

    perm = []
    loc_of = np.full(N_NODES, -1, np.int64)
    core_of = np.full(N_NODES, -1, np.int64)
    for c, (a, b) in enumerate(shards):
        ids = np.arange(a, b)
        ids = ids[np.argsort(-deg[a:b], kind="stable")]
        loc_of[ids] = np.arange(len(ids))
        core_of[ids] = c
        perm.append(np.concatenate([ids, np.full(NB - len(ids), -1, np.int64)]))

    d_t = np.zeros(T, np.int64)
    for c in range(NC):
        for t in range(T):
            ids = perm[c][t * 128 : (t + 1) * 128]
            real = ids[ids >= 0]
            if len(real):
                d_t[t] = max(d_t[t], deg[real].max())
    d_t = np.maximum(d_t, 1).astype(np.int64)

    # global row layout: row = chunk(loc)*NC*CH + core*CH + (loc - chunk*CH)
    def full_row(core, loc):
        ch = loc // CH
        return ch * (NC * CH) + core * CH + (loc - ch * CH)

    assert perm[0][NB - 1] == -1
    DUMMY = int(full_row(0, NB - 1))

    # per-node global row, and node ids in global-row order
    rowmap = np.full(N_NODES, -1, np.int64)
    rows_order = np.full(NC * NB, -1, np.int64)
    loc_all = np.arange(NB)
    for c in range(NC):
        r = full_row(c, loc_all)
        rows_order[r] = perm[c]
        real = perm[c] >= 0
        rowmap[perm[c][real]] = r[real]

    idx16, masks = [], []
    for c in range(NC):
        cols_i, cols_m = [], []
        for t in range(T):
            d = int(d_t[t])
            ids = perm[c][t * 128 : (t + 1) * 128]
            si = np.full((d, 128), DUMMY, np.int64)
            mk = np.full((128, d), -1e9, np.float32)
            for p in range(128):
                g = ids[p]
                if g < 0:
                    mk[p, 0] = 0.0  # keep softmax denom nonzero for padding nodes
                    continue
                srcs = src_by_dst[starts[g] : starts[g + 1]]
                rows = rowmap[srcs]
                si[: len(rows), p] = rows
                mk[p, : len(rows)] = 0.0
            flat = si.reshape(-1)
            cols_i.append(np.tile(flat.reshape(-1, 16).T.astype(np.int16), (8, 1)))
            cols_m.append(mk)
        idx16.append(np.concatenate(cols_i, axis=1))
        masks.append(np.concatenate(cols_m, axis=1).astype(np.float32))

    return dict(NB=NB, T=T, CH=CH, TC=TC, d_t=d_t, perm=perm, idx16=idx16,
                masks=masks, rows_order=rows_order, full_row=full_row)


def _build(NB, T, d_t):
    import concourse.bass as bass
    import concourse.bacc as bacc
    import concourse.mybir as mybir
    import concourse.tile as tile

    f32 = mybir.dt.float32
    bf16 = mybir.dt.bfloat16
    i16 = mybir.dt.int16
    AL = mybir.AluOpType
    AF = mybir.ActivationFunctionType
    AX = mybir.AxisListType

    CH = NB // G_CH
    TC = T // G_CH
    GT = NC * T          # global tiles in the full table
    NR = NC * NB         # full-table rows
    Sd = int(d_t.sum())
    d_off = np.concatenate([[0], np.cumsum(d_t)]).astype(np.int64)

    nc = bacc.Bacc("TRN2", target_bir_lowering=False, debug=False,
                   num_devices=NC, num_swdge_queues=4)

    def inp(name, shape, dt):
        return nc.dram_tensor(name, shape, dt, kind="ExternalInput").ap()

    featT = inp("featT", [P, NR], bf16)          # full table, global-row order
    idx = inp("idx", [P, Sd * 8], i16)
    mask = inp("mask", [P, Sd], f32)
    onehot = inp("onehot", [P, T * G], bf16)
    W_in = inp("W_in", [P, HID], bf16)
    b_in = inp("b_in", [1, HID], bf16)
    W0s = inp("W0s", [P, HID], bf16)             # W_in @ W_src[0]
    W0d = inp("W0d", [P, HID], bf16)
    WsP = inp("WsP", [P, 2 * 2 * HID], bf16)     # layers 1,2 x k-chunks
    WdP = inp("WdP", [P, 2 * 2 * HID], bf16)
    bsP = inp("bsP", [1, LAYERS * HID], bf16)
    bdP = inp("bdP", [1, LAYERS * HID], bf16)
    aT = inp("aT", [P, LAYERS * HID], bf16)
    onescol = inp("onescol", [1, P], bf16)
    ones64 = inp("ones64", [1, G], f32)
    ident = inp("ident", [P, P], f32)
    Wc1P = inp("Wc1P", [P, 4 * P], f32)
    Wc2P = inp("Wc2P", [P, 2 * P], f32)
    Wc3 = inp("Wc3", [P, OUT_DIM], f32)
    bc1 = inp("bc1", [1, HID], f32)
    bc2 = inp("bc2", [1, P], f32)
    bc3 = inp("bc3", [1, OUT_DIM], f32)

    out = nc.dram_tensor("out", [OUT_DIM, G], f32, kind="ExternalOutput").ap()

    # ---- per-core geometry (host-side constants) ----
    # own tile t <-> global tile: g = (t // TC)*(NC*TC) + core*TC + (t % TC)
    # core id is baked per-instance? NO — same program on all cores. The global
    # tile index depends on the core id, which we cannot bake. Instead each
    # core receives its own featT column order? featT is global-row ordered and
    # identical on all cores; own-shard columns differ per core. We pass the
    # own-shard features separately to keep the program core-independent.
    featO = inp("featO", [P, NB], bf16)          # own-shard features, loc order

    with tile.TileContext(nc) as tc, ExitStack() as ctx:
        pers = ctx.enter_context(tc.tile_pool(name="pers", bufs=1))
        big = ctx.enter_context(tc.tile_pool(name="big", bufs=1))
        sm = ctx.enter_context(tc.tile_pool(name="sm", bufs=3))
        psum = ctx.enter_context(tc.tile_pool(name="psum", bufs=2, space="PSUM"))
        dram = ctx.enter_context(tc.tile_pool(name="dram", bufs=1, space="DRAM"))

        _load_engines = [nc.sync, nc.scalar]
        _load_i = [0]

        def load(ap_src, shape, dt, name):
            t = pers.tile(shape, dt, name=name)
            eng = _load_engines[_load_i[0] % len(_load_engines)]
            _load_i[0] += 1
            eng.dma_start(t[:], ap_src)
            return t

        idx_sb = load(idx[:], [P, Sd * 8], i16, "idx_sb")
        mask_sb = load(mask[:], [P, Sd], f32, "mask_sb")
        featO_sb = load(featO[:], [P, NB], bf16, "featO_sb")
        W_in_sb = load(W_in[:], [P, HID], bf16, "W_in_sb")
        b_in_sb = load(b_in[:], [1, HID], bf16, "b_in_sb")
        W0s_sb = load(W0s[:], [P, HID], bf16, "W0s_sb")
        W0d_sb = load(W0d[:], [P, HID], bf16, "W0d_sb")
        Ws_sb = load(WsP[:], [P, 4 * HID], bf16, "Ws_sb")
        Wd_sb = load(WdP[:], [P, 4 * HID], bf16, "Wd_sb")
        bs_sb = load(bsP[:], [1, LAYERS * HID], bf16, "bs_sb")
        bd_sb = load(bdP[:], [1, LAYERS * HID], bf16, "bd_sb")
        a_sb = load(aT[:], [P, LAYERS * HID], bf16, "a_sb")
        ones_sb = load(onescol[:], [1, P], bf16, "ones_sb")
        ones64_sb = load(ones64[:], [1, G], f32, "ones64_sb")
        ident_sb = load(ident[:], [P, P], f32, "ident_sb")
        onehot_sb = load(onehot[:], [P, T * G], bf16, "onehot_sb")
        Wc1_sb = load(Wc1P[:], [P, 4 * P], f32, "Wc1_sb")
        Wc2_sb = load(Wc2P[:], [P, 2 * P], f32, "Wc2_sb")
        Wc3_sb = load(Wc3[:], [P, OUT_DIM], f32, "Wc3_sb")
        bc1_sb = load(bc1[:], [1, HID], f32, "bc1_sb")
        bc2_sb = load(bc2[:], [1, P], f32, "bc2_sb")
        bc3_sb = load(bc3[:], [1, OUT_DIM], f32, "bc3_sb")

        h_sb = pers.tile([P, T * HID], f32, name="h_sb")
        hT_sb = pers.tile([P, 2 * NB], bf16, name="hT_sb")
        fd_sb = [pers.tile([P, T * HID], bf16, name=f"fd_sb{i}") for i in range(2)]

        def hslice(t):
            return h_sb[:, t * HID : (t + 1) * HID]

        def transpose_to_hT(t):
            for k in range(2):
                tp = psum.tile([P, P], f32, tag="tp", space="PSUM")
                nc.tensor.transpose(tp[:], hslice(t)[:, k * P : (k + 1) * P], ident_sb[:])
                nc.vector.tensor_copy(
                    hT_sb[:, k * NB + t * P : k * NB + (t + 1) * P], tp[:])

        # ---- layer-0: full fs table computed locally (replicated) ----
        fs_full0 = dram.tile([NR, HID], bf16, tag="fs_full0", bufs=1)

        STORE_B = 4  # tiles per batched DRAM store
        for g0 in range(0, GT, STORE_B):
            fsx4 = sm.tile([P, STORE_B * HID], bf16, tag="fsx4", bufs=2, name=f"fsx4_{g0}")
            for j in range(STORE_B):
                g = g0 + j
                if g % STREAM_TILES == 0:
                    fstream = big.tile([P, STREAM_TILES * P], bf16, tag="fstream",
                                       bufs=2, name=f"fstream_{g}")
                    nc.sync.dma_start(
                        fstream[:], featT[:, g * P : (g + STREAM_TILES) * P])
                col = (g % STREAM_TILES) * P
                pf = psum.tile([P, HID], f32, tag="mm", space="PSUM")
                nc.tensor.matmul(pf[:], ones_sb[:1, :], bs_sb[:1, 0:HID],
                                 start=True, stop=False)
                nc.tensor.matmul(pf[:], fstream[:, col : col + P], W0s_sb[:],
                                 start=False, stop=True)
                eng = nc.vector if (g % 2 == 0) else nc.scalar
                if g % 2 == 0:
                    nc.vector.tensor_copy(fsx4[:, j * HID : (j + 1) * HID], pf[:])
                else:
                    nc.scalar.copy(fsx4[:, j * HID : (j + 1) * HID], pf[:])
            dst_ap = fs_full0[g0 * P : (g0 + STORE_B) * P, :].rearrange(
                "(c p) f -> p c f", p=P)
            nc.scalar.dma_start(dst_ap, fsx4[:].rearrange("p (c f) -> p c f", f=HID))

        # ---- h0 + fd0 for own shard ----
        for t in range(T):
            ph = psum.tile([P, HID], f32, tag="mm", space="PSUM")
            nc.tensor.matmul(ph[:], ones_sb[:1, :], b_in_sb[:1, :], start=True, stop=False)
            nc.tensor.matmul(ph[:], featO_sb[:, t * P : (t + 1) * P], W_in_sb[:],
                             start=False, stop=True)
            nc.vector.tensor_copy(hslice(t), ph[:])
            pd = psum.tile([P, HID], f32, tag="mm", space="PSUM")
            nc.tensor.matmul(pd[:], ones_sb[:1, :], bd_sb[:1, 0:HID], start=True, stop=False)
            nc.tensor.matmul(pd[:], featO_sb[:, t * P : (t + 1) * P], W0d_sb[:],
                             start=False, stop=True)
            nc.scalar.copy(fd_sb[0][:, t * HID : (t + 1) * HID], pd[:])

        # ---- GAT layers ----
        pool_ps = psum.tile([G, HID], f32, tag="poolps", space="PSUM", bufs=1)

        fs_tables = [fs_full0]
        for l in range(1, LAYERS):
            fs_tables.append(dram.tile([NR, HID], bf16, tag=f"fs_full{l}", bufs=1,
                                       name=f"fs_full{l}"))
        stg = {}
        for l in range(1, LAYERS):
            for q in range(G_CH):
                stg[(l, q)] = dram.tile([CH, HID], bf16, tag=f"stg{l}_{q}", bufs=1,
                                        name=f"stg{l}_{q}")

        for l in range(LAYERS):
            fs_cur = fs_tables[l]
            fd_cur = fd_sb[l % 2]
            fd_nxt = fd_sb[(l + 1) % 2]

            def st0(t):
                d = int(d_t[t])
                io8 = int(d_off[t]) * 8
                fsg = big.tile([P, d, HID], bf16, tag="fsg", bufs=3, name=f"fsg{l}_{t}")
                nc.gpsimd.dma_gather(
                    fsg[:], fs_cur[:],
                    idx_sb[:, io8 : io8 + d * 8],
                    d * P, d * P, HID, queue_num=t % 4,
                    single_packet=False)
                return fsg

            def st1(t, fsg):
                d = int(d_t[t])
                x = big.tile([P, d, HID], bf16, tag="xya", bufs=3, name=f"x{l}_{t}")
                nc.vector.tensor_tensor(
                    x[:], fsg[:],
                    fd_cur[:, t * HID : (t + 1) * HID].unsqueeze(1).to_broadcast(
                        [P, d, HID]),
                    AL.add)
                nc.scalar.activation(x[:], x[:], AF.Prelu, alpha=0.2)
                return x

            def st2(t, x):
                d = int(d_t[t])
                mo = int(d_off[t])
                nc.vector.tensor_tensor(
                    x[:], x[:],
                    a_sb[:, l * HID : (l + 1) * HID].unsqueeze(1).to_broadcast(
                        [P, d, HID]),
                    AL.mult)
                x4 = x[:].rearrange("p d (h k) -> p d h k", h=HEADS)
                n = DH
                while n > 2:
                    n2 = n // 2
                    nc.vector.tensor_tensor(
                        x4[:, :, :, :n2], x4[:, :, :, :n2], x4[:, :, :, n2 : 2 * n2],
                        AL.add)
                    n = n2
                nc.vector.tensor_tensor(
                    x4[:, :, :, 1], x4[:, :, :, 1],
                    mask_sb[:, mo : mo + d].unsqueeze(2).to_broadcast([P, d, HEADS]),
                    AL.add)
                score = sm.tile([P, d, HEADS], f32, tag="score", bufs=2, name=f"sc{l}_{t}")
                nc.vector.tensor_tensor(
                    score[:], x4[:, :, :, 0], x4[:, :, :, 1], AL.add)
                ex = sm.tile([P, d, HEADS], bf16, tag="ex", bufs=2, name=f"ex{l}_{t}")
                nc.scalar.activation(ex[:], score[:], AF.Exp)
                denom = sm.tile([P, HEADS], f32, tag="denom", name=f"dn{l}_{t}")
                nc.vector.tensor_reduce(
                    denom[:], ex[:].rearrange("p d h -> p h d"), axis=AX.X, op=AL.add)
                invd = sm.tile([P, HEADS], f32, tag="invd", name=f"iv{l}_{t}")
                nc.vector.reciprocal(invd[:], denom[:])
                return x, ex, invd

            def st3(t, fsg, x, ex, invd):
                d = int(d_t[t])
                # alpha broadcast (unnormalized): overwrite dead x tile
                nc.scalar.copy(
                    x[:].rearrange("p d (h k) -> p d h k", h=HEADS),
                    ex[:].unsqueeze(3).to_broadcast([P, d, HEADS, DH]))
                nc.vector.tensor_tensor(fsg[:], fsg[:], x[:], AL.mult)
                n = d
                while n > 2:
                    n2 = n // 2
                    nc.vector.tensor_tensor(
                        fsg[:, :n2, :], fsg[:, :n2, :], fsg[:, n2 : 2 * n2, :], AL.add)
                    if n % 2:
                        nc.vector.tensor_tensor(
                            fsg[:, 0, :], fsg[:, 0, :], fsg[:, n - 1, :], AL.add)
                    n = n2
                if n == 2:
                    nc.vector.tensor_tensor(
                        fsg[:, 0, :], fsg[:, 0, :], fsg[:, 1, :], AL.add)
                hnew = sm.tile([P, HID], f32, tag="hnew", bufs=2, name=f"hn{l}_{t}")
                nc.vector.tensor_tensor(
                    hnew[:].rearrange("p (h k) -> p h k", h=HEADS),
                    fsg[:, 0, :].rearrange("p (h k) -> p h k", h=HEADS),
                    invd[:].unsqueeze(2).to_broadcast([P, HEADS, DH]),
                    AL.mult)
                nc.vector.tensor_tensor(hnew[:], hnew[:], hslice(t), AL.add)
                nc.scalar.activation(hslice(t), hnew[:], AF.Relu)
                if l < LAYERS - 1:
                    transpose_to_hT(t)
                    # next-layer projections for this tile
                    lb = l + 1
                    for which, W_t, b_t in (("s", Ws_sb, bs_sb), ("d", Wd_sb, bd_sb)):
                        pf = psum.tile([P, HID], f32, tag="mm", space="PSUM")
                        nc.tensor.matmul(
                            pf[:], ones_sb[:1, :],
                            b_t[:1, lb * HID : (lb + 1) * HID], start=True, stop=False)
                        for k in range(2):
                            nc.tensor.matmul(
                                pf[:],
                                hT_sb[:, k * NB + t * P : k * NB + (t + 1) * P],
                                W_t[:, ((lb - 1) * 2 + k) * HID : ((lb - 1) * 2 + k + 1) * HID],
                                start=False, stop=(k == 1))
                        if which == "s":
                            fsx = sm.tile([P, HID], bf16, tag="fsx", bufs=2, name=f"fsx{l}_{t}")
                            nc.scalar.copy(fsx[:], pf[:])
                            q = t // TC
                            r0 = (t % TC) * P
                            nc.sync.dma_start(stg[(lb, q)][r0 : r0 + P, :], fsx[:])
                        else:
                            nc.scalar.copy(fd_nxt[:, t * HID : (t + 1) * HID], pf[:])
                    if t % TC == TC - 1:
                        q = t // TC
                        nc.gpsimd.collective_compute(
                            "AllGather", AL.bypass, replica_groups=[list(range(NC))],
                            ins=[stg[(lb, q)].opt()],
                            outs=[fs_tables[lb][q * NC * CH : (q + 1) * NC * CH, :]])
                else:
                    hb = sm.tile([P, HID], bf16, tag="hb", bufs=2, name=f"hb{l}_{t}")
                    nc.scalar.copy(hb[:], hslice(t))
                    nc.tensor.matmul(
                        pool_ps[:], onehot_sb[:, t * G : (t + 1) * G], hb[:],
                        start=(t == 0), stop=(t == T - 1))

            live = {}
            for i in range(T + 3):
                if 0 <= i - 3 < T:
                    fsg3, x3, ex3, invd3 = live.pop(i - 3)
                    st3(i - 3, fsg3, x3, ex3, invd3)
                if 0 <= i - 2 < T:
                    fsg2, x2 = live[i - 2][:2]
                    x2, ex2, invd2 = st2(i - 2, x2)
                    live[i - 2] = (fsg2, x2, ex2, invd2)
                if 0 <= i - 1 < T:
                    fsg1 = live[i - 1][0]
                    x = st1(i - 1, fsg1)
                    live[i - 1] = (fsg1, x, None, None)
                if i < T:
                    fsg = st0(i)
                    live[i] = (fsg, None, None, None)

        # ---- pooling allreduce ----
        pool_sb = sm.tile([G, HID], f32, tag="pool_sb", bufs=1)
        nc.vector.tensor_copy(pool_sb[:], pool_ps[:])
        pin = dram.tile([G, HID], f32, tag="pin")
        pout = dram.tile([G, HID], f32, tag="pout", addr_space="Shared")
        nc.sync.dma_start(pin[:], pool_sb[:])
        nc.gpsimd.collective_compute(
            "AllReduce", AL.add, replica_groups=[list(range(NC))],
            ins=[pin.opt()], outs=[pout.opt()])
        pool2 = sm.tile([G, HID], f32, tag="pool2", bufs=1)
        nc.sync.dma_start(pool2[:], pout[:])

        # ---- classifier (f32) ----
        poolT = sm.tile([P, 2 * G], f32, tag="poolT", bufs=1)
        for k in range(2):
            tpp = psum.tile([P, G], f32, tag="cls", space="PSUM")
            nc.tensor.transpose(tpp[:], pool2[:, k * P : (k + 1) * P], ident_sb[:G, :G])
            nc.vector.tensor_copy(poolT[:, k * G : (k + 1) * G], tpp[:])
        x1 = sm.tile([P, 2 * G], f32, tag="x1", bufs=1)
        for p2 in range(2):
            ps1 = psum.tile([P, G], f32, tag="cls", space="PSUM")
            nc.tensor.matmul(ps1[:], bc1_sb[:1, p2 * P : (p2 + 1) * P], ones64_sb[:1, :],
                             start=True, stop=False)
            for k in range(2):
                nc.tensor.matmul(
                    ps1[:], Wc1_sb[:, (k * 2 + p2) * P : (k * 2 + p2 + 1) * P],
                    poolT[:, k * G : (k + 1) * G], start=False, stop=(k == 1))
            nc.scalar.activation(x1[:, p2 * G : (p2 + 1) * G], ps1[:], AF.Relu)
        ps2 = psum.tile([P, G], f32, tag="cls", space="PSUM")
        nc.tensor.matmul(ps2[:], bc2_sb[:1, :], ones64_sb[:1, :], start=True, stop=False)
        for k in range(2):
            nc.tensor.matmul(ps2[:], Wc2_sb[:, k * P : (k + 1) * P],
                             x1[:, k * G : (k + 1) * G], start=False, stop=(k == 1))
        x2 = sm.tile([P, G], f32, tag="x2", bufs=1)
        nc.scalar.activation(x2[:], ps2[:], AF.Relu)
        ps3 = psum.tile([OUT_DIM, G], f32, tag="cls", space="PSUM")
        nc.tensor.matmul(ps3[:], bc3_sb[:1, :], ones64_sb[:1, :], start=True, stop=False)
        nc.tensor.matmul(ps3[:], Wc3_sb[:], x2[:], start=False, stop=True)
        out_sb = sm.tile([OUT_DIM, G], f32, tag="out_sb", bufs=1)
        nc.vector.tensor_copy(out_sb[:], ps3[:])
        nc.sync.dma_start(out[:], out_sb[:])

    nc.compile()
    return nc


def _prep_inputs(inputs, pp):
    NB, T = pp["NB"], pp["T"]
    f = {k: np.asarray(v) for k, v in inputs.items()}

    def bf(x):
        return np.ascontiguousarray(np.asarray(x, np.float32).astype(BF))

    W_in_f = np.asarray(f["W_in"], np.float32)
    b_in_f = np.asarray(f["b_in"], np.float32)
    Ws0 = np.asarray(f["W_src"][0], np.float32)
    Wd0 = np.asarray(f["W_dst"][0], np.float32)

    def pack_w12(W):
        # layers 1,2: [2 layers x 2 k-chunks][128, 256]
        blocks = []
        for l in (1, 2):
            Wl = np.asarray(W[l], np.float32)
            blocks.extend([Wl[:P], Wl[P:]])
        return np.concatenate(blocks, axis=1)

    aT = np.concatenate([np.tile(f["attn"][l].reshape(1, HID), (P, 1))
                         for l in range(LAYERS)], axis=1).astype(np.float32)
    Wc1 = np.asarray(f["Wc1"], np.float32)
    Wc1P = np.concatenate([Wc1[128 * k : 128 * (k + 1), 128 * p2 : 128 * (p2 + 1)]
                           for k in range(2) for p2 in range(2)], axis=1)
    Wc2 = np.asarray(f["Wc2"], np.float32)
    Wc2P = np.concatenate([Wc2[128 * k : 128 * (k + 1), :] for k in range(2)], axis=1)

    feature = np.asarray(f["feature"], np.float32)
    rows = pp["rows_order"]
    featT_full = np.zeros((NC * NB, IN_DIM), np.float32)
    real = rows >= 0
    featT_full[real] = feature[rows[real]]
    featT_full = np.ascontiguousarray(featT_full.T.astype(BF))

    shared = {
        "featT": featT_full,
        "W_in": bf(W_in_f), "b_in": bf(b_in_f).reshape(1, HID),
        "W0s": bf(W_in_f @ Ws0), "W0d": bf(W_in_f @ Wd0),
        "WsP": bf(pack_w12(f["W_src"])), "WdP": bf(pack_w12(f["W_dst"])),
        "bsP": bf(np.concatenate(
            [(b_in_f @ Ws0 + f["b_src"][0])[None]]
            + [np.asarray(f["b_src"][l], np.float32)[None] for l in range(1, LAYERS)]
        )).reshape(1, LAYERS * HID),
        "bdP": bf(np.concatenate(
            [(b_in_f @ Wd0 + f["b_dst"][0])[None]]
            + [np.asarray(f["b_dst"][l], np.float32)[None] for l in range(1, LAYERS)]
        )).reshape(1, LAYERS * HID),
        "aT": bf(aT),
        "onescol": np.ones((1, P), BF),
        "ones64": np.ones((1, G), np.float32),
        "ident": np.eye(P, dtype=np.float32),
        "Wc1P": np.ascontiguousarray(Wc1P),
        "Wc2P": np.ascontiguousarray(Wc2P),
        "Wc3": np.ascontiguousarray(np.asarray(f["Wc3"], np.float32)),
        "bc1": np.asarray(f["bc1"], np.float32).reshape(1, HID),
        "bc2": np.asarray(f["bc2"], np.float32).reshape(1, P),
        "bc3": np.asarray(f["bc3"], np.float32).reshape(1, OUT_DIM),
    }

    gids = np.asarray(f["graph_ids"], np.int64)
    in_maps = []
    for c in range(NC):
        ids = pp["perm"][c]
        real = ids >= 0
        feat = np.zeros((NB, IN_DIM), np.float32)
        feat[real] = feature[ids[real]]
        oh = np.zeros((NB, G), np.float32)
        oh[np.nonzero(real)[0], gids[ids[real]]] = 1.0
        oh = oh.reshape(NB // P, P, G).transpose(1, 0, 2).reshape(P, -1)
        m = dict(shared)
        m["featO"] = np.ascontiguousarray(feat.T.astype(BF))
        m["idx"] = np.ascontiguousarray(pp["idx16"][c])
        m["mask"] = np.ascontiguousarray(pp["masks"][c])
        m["onehot"] = np.ascontiguousarray(oh.astype(BF))
        in_maps.append(m)
    return in_maps


def kernel(**inputs):
    from concourse import bass_utils

    src = np.asarray(inputs["src"], np.int64)
    dst = np.asarray(inputs["dst"], np.int64)

    key = (src[:16].tobytes(), dst[:16].tobytes())
    state = _CACHE.get(key)
    if state is None:
        pp = _preprocess(src, dst)
        nc = _build(pp["NB"], pp["T"], pp["d_t"])
        state = (pp, nc)
        _CACHE[key] = state
    pp, nc = state

    in_maps = _prep_inputs(inputs, pp)
    res = bass_utils.run_bass_kernel_spmd(nc, in_maps, core_ids=list(range(NC)))
    return np.ascontiguousarray(res.results[0]["out"].T.astype(np.float32))


# revision 10
# speedup vs baseline: 1.4952x; 1.4952x over previous
"""GATv2 message-passing network (3 layers + sum-pool + MLP) on 8 trn2 NeuronCores.

Strategy: shard dst-nodes across 8 cores (contiguous ranges balanced by edge
count). Layer-0 source projections are computed REPLICATED on every core from
the full input feature matrix (PE is idle, features are only 5MB) — no
layer-0 AllGather. For layers 1-2 the fs table is AllGathered in G_CH chunks
that are software-pipelined into the previous layer's edge loop: as soon as a
chunk's worth of dst-tiles finish their attention update, their next-layer
projections run on the PE and that chunk's AllGather fires, hiding collective
latency behind the (DVE-bound) edge-softmax compute. Per-edge work runs in a
[128 dst, d, 256] layout: gather fs rows (one SWDGE gather per tile), add fd
broadcast, Prelu, a-weighted head reduce, edge softmax (normalization folded
to the end), alpha-weighted value sum. Pooling is a one-hot matmul into PSUM
+ AllReduce; the classifier is replicated on every core in f32.
"""
import sys
from contextlib import ExitStack

sys.path.insert(0, "/opt/trn_rl_repo")

import numpy as np
import ml_dtypes

BF = ml_dtypes.bfloat16
NC = 8
N_NODES = 20000
N_EDGES = 320000
IN_DIM = 128
HID = 256
HEADS = 8
DH = 32
LAYERS = 3
G = 64
OUT_DIM = 10
P = 128

G_CH = 4          # AllGather chunks per layer
NQ_GATHER = 4     # SWDGE queues per tile gather
STREAM_TILES = 8   # featT streaming chunk (global tiles per DMA)

_CACHE = {}


def _preprocess(src, dst):
    deg = np.bincount(dst, minlength=N_NODES)
    order = np.argsort(dst, kind="stable")
    src_by_dst = src[order]
    starts = np.zeros(N_NODES + 1, np.int64)
    np.cumsum(deg, out=starts[1:])

    csum = starts[1:]
    bounds = [0]
    for c in range(1, NC):
        i = int(np.searchsorted(csum, N_EDGES * c / NC))
        bounds.append(i + 1)
    bounds.append(N_NODES)
    shards = [(bounds[i], bounds[i + 1]) for i in range(NC)]
    node_counts = [b - a for a, b in shards]

    NB = ((max(node_counts) + 1 + 127) // 128) * 128
    while NB % (G_CH * 128):
        NB += 128
    T = NB // 128
    CH = NB // G_CH
    TC = T // G_CH

    perm = []
    loc_of = np.full(N_NODES, -1, np.int64)
    core_of = np.full(N_NODES, -1, np.int64)
    for c, (a, b) in enumerate(shards):
        ids = np.arange(a, b)
        ids = ids[np.argsort(-deg[a:b], kind="stable")]
        loc_of[ids] = np.arange(len(ids))
        core_of[ids] = c
        perm.append(np.concatenate([ids, np.full(NB - len(ids), -1, np.int64)]))

    d_t = np.zeros(T, np.int64)
    for c in range(NC):
        for t in range(T):
            ids = perm[c][t * 128 : (t + 1) * 128]
            real = ids[ids >= 0]
            if len(real):
                d_t[t] = max(d_t[t], deg[real].max())
    d_t = np.maximum(d_t, 1).astype(np.int64)

    # global row layout: row = chunk(loc)*NC*CH + core*CH + (loc - chunk*CH)
    def full_row(core, loc):
        ch = loc // CH
        return ch * (NC * CH) + core * CH + (loc - ch * CH)

    assert perm[0][NB - 1] == -1
    DUMMY = int(full_row(0, NB - 1))

    # per-node global row, and node ids in global-row order
    rowmap = np.full(N_NODES, -1, np.int64)
    rows_order = np.full(NC * NB, -1, np.int64)
    loc_all = np.arange(NB)
    for c in range(NC):
        r = full_row(c, loc_all)
        rows_order[r] = perm[c]
        real = perm[c] >= 0
        rowmap[perm[c][real]] = r[real]

    idx16, masks = [], []
    for c in range(NC):
        cols_i, cols_m = [], []
        for t in range(T):
            d = int(d_t[t])
            ids = perm[c][t * 128 : (t + 1) * 128]
            si = np.full((d, 128), DUMMY, np.int64)
            mk = np.full((128, d), -1e9, np.float32)
            for p in range(128):
                g = ids[p]
                if g < 0:
                    mk[p, 0] = 0.0  # keep softmax denom nonzero for padding nodes
                    continue
                srcs = src_by_dst[starts[g] : starts[g + 1]]
                rows = rowmap[srcs]
                si[: len(rows), p] = rows
                mk[p, : len(rows)] = 0.0
            flat = si.reshape(-1)
            cols_i.append(np.tile(flat.reshape(-1, 16).T.astype(np.int16), (8, 1)))
            cols_m.append(mk)
        idx16.append(np.concatenate(cols_i, axis=1))
        masks.append(np.concatenate(cols_m, axis=1).astype(np.float32))

    return dict(NB=NB, T=T, CH=CH, TC=TC, d_t=d_t, perm=perm, idx16=idx16,
                masks=masks, rows_order=rows_order, full_row=full_row)


def _build(NB, T, d_t):
    import concourse.bass as bass
    import concourse.bacc as bacc
    import concourse.mybir as mybir
    import concourse.tile as tile

    f32 = mybir.dt.float32
    bf16 = mybir.dt.bfloat16
    i16 = mybir.dt.int16
    AL = mybir.AluOpType
    AF = mybir.ActivationFunctionType
    AX = mybir.AxisListType

    CH = NB // G_CH
    TC = T // G_CH
    GT = NC * T          # global tiles in the full table
    NR = NC * NB         # full-table rows
    Sd = int(d_t.sum())
    d_off = np.concatenate([[0], np.cumsum(d_t)]).astype(np.int64)

    nc = bacc.Bacc("TRN2", target_bir_lowering=False, debug=False,
                   num_devices=NC, num_swdge_queues=4)

    def inp(name, shape, dt):
        return nc.dram_tensor(name, shape, dt, kind="ExternalInput").ap()

    featT = inp("featT", [P, NR], bf16)          # full table, global-row order
    idx = inp("idx", [P, Sd * 8], i16)
    mask = inp("mask", [P, Sd], f32)
    onehot = inp("onehot", [P, T * G], bf16)
    W_in = inp("W_in", [P, HID], bf16)
    b_in = inp("b_in", [1, HID], bf16)
    W0s = inp("W0s", [P, HID], bf16)             # W_in @ W_src[0]
    W0d = inp("W0d", [P, HID], bf16)
    WsP = inp("WsP", [P, 2 * 2 * HID], bf16)     # layers 1,2 x k-chunks
    WdP = inp("WdP", [P, 2 * 2 * HID], bf16)
    bsP = inp("bsP", [1, LAYERS * HID], bf16)
    bdP = inp("bdP", [1, LAYERS * HID], bf16)
    aT = inp("aT", [P, LAYERS * HID], bf16)
    onescol = inp("onescol", [1, P], bf16)
    ones64 = inp("ones64", [1, G], f32)
    ident = inp("ident", [P, P], f32)
    Wc1P = inp("Wc1P", [P, 4 * P], f32)
    Wc2P = inp("Wc2P", [P, 2 * P], f32)
    Wc3 = inp("Wc3", [P, OUT_DIM], f32)
    bc1 = inp("bc1", [1, HID], f32)
    bc2 = inp("bc2", [1, P], f32)
    bc3 = inp("bc3", [1, OUT_DIM], f32)

    out = nc.dram_tensor("out", [OUT_DIM, G], f32, kind="ExternalOutput").ap()

    # ---- per-core geometry (host-side constants) ----
    # own tile t <-> global tile: g = (t // TC)*(NC*TC) + core*TC + (t % TC)
    # core id is baked per-instance? NO — same program on all cores. The global
    # tile index depends on the core id, which we cannot bake. Instead each
    # core receives its own featT column order? featT is global-row ordered and
    # identical on all cores; own-shard columns differ per core. We pass the
    # own-shard features separately to keep the program core-independent.
    featO = inp("featO", [P, NB], bf16)          # own-shard features, loc order

    with tile.TileContext(nc) as tc, ExitStack() as ctx:
        pers = ctx.enter_context(tc.tile_pool(name="pers", bufs=1))
        big = ctx.enter_context(tc.tile_pool(name="big", bufs=1))
        sm = ctx.enter_context(tc.tile_pool(name="sm", bufs=3))
        psum = ctx.enter_context(tc.tile_pool(name="psum", bufs=2, space="PSUM"))
        dram = ctx.enter_context(tc.tile_pool(name="dram", bufs=1, space="DRAM"))

        _load_engines = [nc.sync, nc.scalar]
        _load_i = [0]

        def load(ap_src, shape, dt, name):
            t = pers.tile(shape, dt, name=name)
            eng = _load_engines[_load_i[0] % len(_load_engines)]
            _load_i[0] += 1
            eng.dma_start(t[:], ap_src)
            return t

        idx_sb = load(idx[:], [P, Sd * 8], i16, "idx_sb")
        mask_sb = load(mask[:], [P, Sd], f32, "mask_sb")
        featO_sb = load(featO[:], [P, NB], bf16, "featO_sb")
        W_in_sb = load(W_in[:], [P, HID], bf16, "W_in_sb")
        b_in_sb = load(b_in[:], [1, HID], bf16, "b_in_sb")
        W0s_sb = load(W0s[:], [P, HID], bf16, "W0s_sb")
        W0d_sb = load(W0d[:], [P, HID], bf16, "W0d_sb")
        Ws_sb = load(WsP[:], [P, 4 * HID], bf16, "Ws_sb")
        Wd_sb = load(WdP[:], [P, 4 * HID], bf16, "Wd_sb")
        bs_sb = load(bsP[:], [1, LAYERS * HID], bf16, "bs_sb")
        bd_sb = load(bdP[:], [1, LAYERS * HID], bf16, "bd_sb")
        a_sb = load(aT[:], [P, LAYERS * HID], bf16, "a_sb")
        ones_sb = load(onescol[:], [1, P], bf16, "ones_sb")
        ones64_sb = load(ones64[:], [1, G], f32, "ones64_sb")
        ident_sb = load(ident[:], [P, P], f32, "ident_sb")
        onehot_sb = load(onehot[:], [P, T * G], bf16, "onehot_sb")
        Wc1_sb = load(Wc1P[:], [P, 4 * P], f32, "Wc1_sb")
        Wc2_sb = load(Wc2P[:], [P, 2 * P], f32, "Wc2_sb")
        Wc3_sb = load(Wc3[:], [P, OUT_DIM], f32, "Wc3_sb")
        bc1_sb = load(bc1[:], [1, HID], f32, "bc1_sb")
        bc2_sb = load(bc2[:], [1, P], f32, "bc2_sb")
        bc3_sb = load(bc3[:], [1, OUT_DIM], f32, "bc3_sb")

        h_sb = pers.tile([P, T * HID], f32, name="h_sb")
        hT_sb = pers.tile([P, 2 * NB], bf16, name="hT_sb")
        fd_sb = [pers.tile([P, T * HID], bf16, name=f"fd_sb{i}") for i in range(2)]

        def hslice(t):
            return h_sb[:, t * HID : (t + 1) * HID]

        def transpose_to_hT(t):
            for k in range(2):
                tp = psum.tile([P, P], f32, tag="tp", space="PSUM")
                nc.tensor.transpose(tp[:], hslice(t)[:, k * P : (k + 1) * P], ident_sb[:])
                nc.vector.tensor_copy(
                    hT_sb[:, k * NB + t * P : k * NB + (t + 1) * P], tp[:])

        # ---- layer-0: full fs table computed locally (replicated) ----
        fs_full0 = dram.tile([NR, HID], bf16, tag="fs_full0", bufs=1)

        STORE_B = 4  # tiles per batched DRAM store
        for g0 in range(0, GT, STORE_B):
            fsx4 = sm.tile([P, STORE_B * HID], bf16, tag="fsx4", bufs=2, name=f"fsx4_{g0}")
            for j in range(STORE_B):
                g = g0 + j
                if g % STREAM_TILES == 0:
                    fstream = big.tile([P, STREAM_TILES * P], bf16, tag="fstream",
                                       bufs=2, name=f"fstream_{g}")
                    nc.sync.dma_start(
                        fstream[:], featT[:, g * P : (g + STREAM_TILES) * P])
                col = (g % STREAM_TILES) * P
                pf = psum.tile([P, HID], f32, tag="mm", space="PSUM")
                nc.tensor.matmul(pf[:], ones_sb[:1, :], bs_sb[:1, 0:HID],
                                 start=True, stop=False)
                nc.tensor.matmul(pf[:], fstream[:, col : col + P], W0s_sb[:],
                                 start=False, stop=True)
                eng = nc.vector if (g % 2 == 0) else nc.scalar
                if g % 2 == 0:
                    nc.vector.tensor_copy(fsx4[:, j * HID : (j + 1) * HID], pf[:])
                else:
                    nc.scalar.copy(fsx4[:, j * HID : (j + 1) * HID], pf[:])
            dst_ap = fs_full0[g0 * P : (g0 + STORE_B) * P, :].rearrange(
                "(c p) f -> p c f", p=P)
            nc.scalar.dma_start(dst_ap, fsx4[:].rearrange("p (c f) -> p c f", f=HID))

        # ---- h0 + fd0 for own shard ----
        for t in range(T):
            ph = psum.tile([P, HID], f32, tag="mm", space="PSUM")
            nc.tensor.matmul(ph[:], ones_sb[:1, :], b_in_sb[:1, :], start=True, stop=False)
            nc.tensor.matmul(ph[:], featO_sb[:, t * P : (t + 1) * P], W_in_sb[:],
                             start=False, stop=True)
            nc.vector.tensor_copy(hslice(t), ph[:])
            pd = psum.tile([P, HID], f32, tag="mm", space="PSUM")
            nc.tensor.matmul(pd[:], ones_sb[:1, :], bd_sb[:1, 0:HID], start=True, stop=False)
            nc.tensor.matmul(pd[:], featO_sb[:, t * P : (t + 1) * P], W0d_sb[:],
                             start=False, stop=True)
            nc.scalar.copy(fd_sb[0][:, t * HID : (t + 1) * HID], pd[:])

        # ---- GAT layers ----
        pool_ps = psum.tile([G, HID], f32, tag="poolps", space="PSUM", bufs=1)

        fs_tables = [fs_full0]
        for l in range(1, LAYERS):
            fs_tables.append(dram.tile([NR, HID], bf16, tag=f"fs_full{l}", bufs=1,
                                       name=f"fs_full{l}"))
        stg = {}
        for l in range(1, LAYERS):
            for q in range(G_CH):
                stg[(l, q)] = dram.tile([CH, HID], bf16, tag=f"stg{l}_{q}", bufs=1,
                                        name=f"stg{l}_{q}")

        for l in range(LAYERS):
            fs_cur = fs_tables[l]
            fd_cur = fd_sb[l % 2]
            fd_nxt = fd_sb[(l + 1) % 2]

            def st0(t):
                d = int(d_t[t])
                io8 = int(d_off[t]) * 8
                fsg = big.tile([P, d, HID], bf16, tag="fsg", bufs=3, name=f"fsg{l}_{t}")
                nq = min(NQ_GATHER, d)
                bounds = [round(j * d / nq) for j in range(nq + 1)]
                for j in range(nq):
                    a, b = bounds[j], bounds[j + 1]
                    nc.gpsimd.dma_gather(
                        fsg[:, a:b, :], fs_cur[:],
                        idx_sb[:, io8 + a * 8 : io8 + b * 8],
                        (b - a) * P, (b - a) * P, HID, queue_num=j,
                        single_packet=False)
                return fsg

            def st1(t, fsg):
                d = int(d_t[t])
                x = big.tile([P, d, HID], bf16, tag="xya", bufs=3, name=f"x{l}_{t}")
                nc.vector.tensor_tensor(
                    x[:], fsg[:],
                    fd_cur[:, t * HID : (t + 1) * HID].unsqueeze(1).to_broadcast(
                        [P, d, HID]),
                    AL.add)
                nc.scalar.activation(x[:], x[:], AF.Prelu, alpha=0.2)
                return x

            def st2(t, x):
                d = int(d_t[t])
                mo = int(d_off[t])
                nc.vector.tensor_tensor(
                    x[:], x[:],
                    a_sb[:, l * HID : (l + 1) * HID].unsqueeze(1).to_broadcast(
                        [P, d, HID]),
                    AL.mult)
                x4 = x[:].rearrange("p d (h k) -> p d h k", h=HEADS)
                n = DH
                while n > 2:
                    n2 = n // 2
                    nc.vector.tensor_tensor(
                        x4[:, :, :, :n2], x4[:, :, :, :n2], x4[:, :, :, n2 : 2 * n2],
                        AL.add)
                    n = n2
                nc.vector.tensor_tensor(
                    x4[:, :, :, 1], x4[:, :, :, 1],
                    mask_sb[:, mo : mo + d].unsqueeze(2).to_broadcast([P, d, HEADS]),
                    AL.add)
                score = sm.tile([P, d, HEADS], f32, tag="score", bufs=2, name=f"sc{l}_{t}")
                nc.vector.tensor_tensor(
                    score[:], x4[:, :, :, 0], x4[:, :, :, 1], AL.add)
                ex = sm.tile([P, d, HEADS], bf16, tag="ex", bufs=2, name=f"ex{l}_{t}")
                nc.scalar.activation(ex[:], score[:], AF.Exp)
                denom = sm.tile([P, HEADS], f32, tag="denom", name=f"dn{l}_{t}")
                nc.vector.tensor_reduce(
                    denom[:], ex[:].rearrange("p d h -> p h d"), axis=AX.X, op=AL.add)
                invd = sm.tile([P, HEADS], f32, tag="invd", name=f"iv{l}_{t}")
                nc.vector.reciprocal(invd[:], denom[:])
                return x, ex, invd

            def st3(t, fsg, x, ex, invd):
                d = int(d_t[t])
                # alpha broadcast (unnormalized): overwrite dead x tile
                nc.scalar.copy(
                    x[:].rearrange("p d (h k) -> p d h k", h=HEADS),
                    ex[:].unsqueeze(3).to_broadcast([P, d, HEADS, DH]))
                nc.vector.tensor_tensor(fsg[:], fsg[:], x[:], AL.mult)
                n = d
                while n > 2:
                    n2 = n // 2
                    nc.vector.tensor_tensor(
                        fsg[:, :n2, :], fsg[:, :n2, :], fsg[:, n2 : 2 * n2, :], AL.add)
                    if n % 2:
                        nc.vector.tensor_tensor(
                            fsg[:, 0, :], fsg[:, 0, :], fsg[:, n - 1, :], AL.add)
                    n = n2
                if n == 2:
                    nc.vector.tensor_tensor(
                        fsg[:, 0, :], fsg[:, 0, :], fsg[:, 1, :], AL.add)
                hnew = sm.tile([P, HID], f32, tag="hnew", bufs=2, name=f"hn{l}_{t}")
                nc.vector.tensor_tensor(
                    hnew[:].rearrange("p (h k) -> p h k", h=HEADS),
                    fsg[:, 0, :].rearrange("p (h k) -> p h k", h=HEADS),
                    invd[:].unsqueeze(2).to_broadcast([P, HEADS, DH]),
                    AL.mult)
                nc.vector.tensor_tensor(hnew[:], hnew[:], hslice(t), AL.add)
                nc.scalar.activation(hslice(t), hnew[:], AF.Relu)
                if l < LAYERS - 1:
                    transpose_to_hT(t)
                    # next-layer projections for this tile
                    lb = l + 1
                    for which, W_t, b_t in (("s", Ws_sb, bs_sb), ("d", Wd_sb, bd_sb)):
                        pf = psum.tile([P, HID], f32, tag="mm", space="PSUM")
                        nc.tensor.matmul(
                            pf[:], ones_sb[:1, :],
                            b_t[:1, lb * HID : (lb + 1) * HID], start=True, stop=False)
                        for k in range(2):
                            nc.tensor.matmul(
                                pf[:],
                                hT_sb[:, k * NB + t * P : k * NB + (t + 1) * P],
                                W_t[:, ((lb - 1) * 2 + k) * HID : ((lb - 1) * 2 + k + 1) * HID],
                                start=False, stop=(k == 1))
                        if which == "s":
                            fsx = sm.tile([P, HID], bf16, tag="fsx", bufs=2, name=f"fsx{l}_{t}")
                            nc.scalar.copy(fsx[:], pf[:])
                            q = t // TC
                            r0 = (t % TC) * P
                            nc.sync.dma_start(stg[(lb, q)][r0 : r0 + P, :], fsx[:])
                        else:
                            nc.scalar.copy(fd_nxt[:, t * HID : (t + 1) * HID], pf[:])
                    if t % TC == TC - 1:
                        q = t // TC
                        nc.gpsimd.collective_compute(
                            "AllGather", AL.bypass, replica_groups=[list(range(NC))],
                            ins=[stg[(lb, q)].opt()],
                            outs=[fs_tables[lb][q * NC * CH : (q + 1) * NC * CH, :]])
                else:
                    hb = sm.tile([P, HID], bf16, tag="hb", bufs=2, name=f"hb{l}_{t}")
                    nc.scalar.copy(hb[:], hslice(t))
                    nc.tensor.matmul(
                        pool_ps[:], onehot_sb[:, t * G : (t + 1) * G], hb[:],
                        start=(t == 0), stop=(t == T - 1))

            live = {}
            for i in range(T + 3):
                if 0 <= i - 3 < T:
                    fsg3, x3, ex3, invd3 = live.pop(i - 3)
                    st3(i - 3, fsg3, x3, ex3, invd3)
                if 0 <= i - 2 < T:
                    fsg2, x2 = live[i - 2][:2]
                    x2, ex2, invd2 = st2(i - 2, x2)
                    live[i - 2] = (fsg2, x2, ex2, invd2)
                if 0 <= i - 1 < T:
                    fsg1 = live[i - 1][0]
                    x = st1(i - 1, fsg1)
                    live[i - 1] = (fsg1, x, None, None)
                if i < T:
                    fsg = st0(i)
                    live[i] = (fsg, None, None, None)

        # ---- pooling allreduce ----
        pool_sb = sm.tile([G, HID], f32, tag="pool_sb", bufs=1)
        nc.vector.tensor_copy(pool_sb[:], pool_ps[:])
        pin = dram.tile([G, HID], f32, tag="pin")
        pout = dram.tile([G, HID], f32, tag="pout", addr_space="Shared")
        nc.sync.dma_start(pin[:], pool_sb[:])
        nc.gpsimd.collective_compute(
            "AllReduce", AL.add, replica_groups=[list(range(NC))],
            ins=[pin.opt()], outs=[pout.opt()])
        pool2 = sm.tile([G, HID], f32, tag="pool2", bufs=1)
        nc.sync.dma_start(pool2[:], pout[:])

        # ---- classifier (f32) ----
        poolT = sm.tile([P, 2 * G], f32, tag="poolT", bufs=1)
        for k in range(2):
            tpp = psum.tile([P, G], f32, tag="cls", space="PSUM")
            nc.tensor.transpose(tpp[:], pool2[:, k * P : (k + 1) * P], ident_sb[:G, :G])
            nc.vector.tensor_copy(poolT[:, k * G : (k + 1) * G], tpp[:])
        x1 = sm.tile([P, 2 * G], f32, tag="x1", bufs=1)
        for p2 in range(2):
            ps1 = psum.tile([P, G], f32, tag="cls", space="PSUM")
            nc.tensor.matmul(ps1[:], bc1_sb[:1, p2 * P : (p2 + 1) * P], ones64_sb[:1, :],
                             start=True, stop=False)
            for k in range(2):
                nc.tensor.matmul(
                    ps1[:], Wc1_sb[:, (k * 2 + p2) * P : (k * 2 + p2 + 1) * P],
                    poolT[:, k * G : (k + 1) * G], start=False, stop=(k == 1))
            nc.scalar.activation(x1[:, p2 * G : (p2 + 1) * G], ps1[:], AF.Relu)
        ps2 = psum.tile([P, G], f32, tag="cls", space="PSUM")
        nc.tensor.matmul(ps2[:], bc2_sb[:1, :], ones64_sb[:1, :], start=True, stop=False)
        for k in range(2):
            nc.tensor.matmul(ps2[:], Wc2_sb[:, k * P : (k + 1) * P],
                             x1[:, k * G : (k + 1) * G], start=False, stop=(k == 1))
        x2 = sm.tile([P, G], f32, tag="x2", bufs=1)
        nc.scalar.activation(x2[:], ps2[:], AF.Relu)
        ps3 = psum.tile([OUT_DIM, G], f32, tag="cls", space="PSUM")
        nc.tensor.matmul(ps3[:], bc3_sb[:1, :], ones64_sb[:1, :], start=True, stop=False)
        nc.tensor.matmul(ps3[:], Wc3_sb[:], x2[:], start=False, stop=True)
        out_sb = sm.tile([OUT_DIM, G], f32, tag="out_sb", bufs=1)
        nc.vector.tensor_copy(out_sb[:], ps3[:])
        nc.sync.dma_start(out[:], out_sb[:])

    nc.compile()
    return nc


def _prep_inputs(inputs, pp):
    NB, T = pp["NB"], pp["T"]
    f = {k: np.asarray(v) for k, v in inputs.items()}

    def bf(x):
        return np.ascontiguousarray(np.asarray(x, np.float32).astype(BF))

    W_in_f = np.asarray(f["W_in"], np.float32)
    b_in_f = np.asarray(f["b_in"], np.float32)
    Ws0 = np.asarray(f["W_src"][0], np.float32)
    Wd0 = np.asarray(f["W_dst"][0], np.float32)

    def pack_w12(W):
        # layers 1,2: [2 layers x 2 k-chunks][128, 256]
        blocks = []
        for l in (1, 2):
            Wl = np.asarray(W[l], np.float32)
            blocks.extend([Wl[:P], Wl[P:]])
        return np.concatenate(blocks, axis=1)

    aT = np.concatenate([np.tile(f["attn"][l].reshape(1, HID), (P, 1))
                         for l in range(LAYERS)], axis=1).astype(np.float32)
    Wc1 = np.asarray(f["Wc1"], np.float32)
    Wc1P = np.concatenate([Wc1[128 * k : 128 * (k + 1), 128 * p2 : 128 * (p2 + 1)]
                           for k in range(2) for p2 in range(2)], axis=1)
    Wc2 = np.asarray(f["Wc2"], np.float32)
    Wc2P = np.concatenate([Wc2[128 * k : 128 * (k + 1), :] for k in range(2)], axis=1)

    feature = np.asarray(f["feature"], np.float32)
    rows = pp["rows_order"]
    featT_full = np.zeros((NC * NB, IN_DIM), np.float32)
    real = rows >= 0
    featT_full[real] = feature[rows[real]]
    featT_full = np.ascontiguousarray(featT_full.T.astype(BF))

    shared = {
        "featT": featT_full,
        "W_in": bf(W_in_f), "b_in": bf(b_in_f).reshape(1, HID),
        "W0s": bf(W_in_f @ Ws0), "W0d": bf(W_in_f @ Wd0),
        "WsP": bf(pack_w12(f["W_src"])), "WdP": bf(pack_w12(f["W_dst"])),
        "bsP": bf(np.concatenate(
            [(b_in_f @ Ws0 + f["b_src"][0])[None]]
            + [np.asarray(f["b_src"][l], np.float32)[None] for l in range(1, LAYERS)]
        )).reshape(1, LAYERS * HID),
        "bdP": bf(np.concatenate(
            [(b_in_f @ Wd0 + f["b_dst"][0])[None]]
            + [np.asarray(f["b_dst"][l], np.float32)[None] for l in range(1, LAYERS)]
        )).reshape(1, LAYERS * HID),
        "aT": bf(aT),
        "onescol": np.ones((1, P), BF),
        "ones64": np.ones((1, G), np.float32),
        "ident": np.eye(P, dtype=np.float32),
        "Wc1P": np.ascontiguousarray(Wc1P),
        "Wc2P": np.ascontiguousarray(Wc2P),
        "Wc3": np.ascontiguousarray(np.asarray(f["Wc3"], np.float32)),
        "bc1": np.asarray(f["bc1"], np.float32).reshape(1, HID),
        "bc2": np.asarray(f["bc2"], np.float32).reshape(1, P),
        "bc3": np.asarray(f["bc3"], np.float32).reshape(1, OUT_DIM),
    }

    gids = np.asarray(f["graph_ids"], np.int64)
    in_maps = []
    for c in range(NC):
        ids = pp["perm"][c]
        real = ids >= 0
        feat = np.zeros((NB, IN_DIM), np.float32)
        feat[real] = feature[ids[real]]
        oh = np.zeros((NB, G), np.float32)
        oh[np.nonzero(real)[0], gids[ids[real]]] = 1.0
        oh = oh.reshape(NB // P, P, G).transpose(1, 0, 2).reshape(P, -1)
        m = dict(shared)
        m["featO"] = np.ascontiguousarray(feat.T.astype(BF))
        m["idx"] = np.ascontiguousarray(pp["idx16"][c])
        m["mask"] = np.ascontiguousarray(pp["masks"][c])
        m["onehot"] = np.ascontiguousarray(oh.astype(BF))
        in_maps.append(m)
    return in_maps


def kernel(**inputs):
    from concourse import bass_utils

    src = np.asarray(inputs["src"], np.int64)
    dst = np.asarray(inputs["dst"], np.int64)

    key = (src[:16].tobytes(), dst[:16].tobytes())
    state = _CACHE.get(key)
    if state is None:
        pp = _preprocess(src, dst)
        nc = _build(pp["NB"], pp["T"], pp["d_t"])
        state = (pp, nc)
        _CACHE[key] = state
    pp, nc = state

    in_maps = _prep_inputs(inputs, pp)
    res = bass_utils.run_bass_kernel_spmd(nc, in_maps, core_ids=list(range(NC)))
    return np.ascontiguousarray(res.results[0]["out"].T.astype(np.float32))
